# revision 35
# baseline (speedup 1.0000x reference)
"""Butterworth bandpass (cascaded biquad IIR) Trainium2 kernel.

Problem: y = sosfilt(sos, x) over x[32, 64, 4096] fp32 -- 2048 independent
signals, 4 cascaded DF2T biquads, sequential over T=4096.

Strategy (block FIR with exact 512-tap diagonal-band operators, bf16):
  The cascade is a linear state-space system (A[8,8], B, C, D) whose impulse
  response h decays below 1.2e-3 (l1) past 512 taps, so the IIR is computed
  as a 512-tap block FIR: split T into 128-step blocks; the table
  THW[128, 512] holds the four exact skew-diagonal operator blocks
  T_d[u, c] = h[c - u + 128 d] (d = 0..3, from the state-space form
  Z A_L^{d-1} F), and output block o accumulates T_{o-j} @ x_j for the seven
  preceding/current input blocks j.  All device work is TensorE matmuls over
  [signal, time] tiles in bf16 (1 cyc/row; fp32 PSUM accumulation):
    - the host pre-transposes x into xT block layout [128 time, w, r, sig],
      so the device does no transposes and input DMAs are flat contiguous
      2KB-per-partition lines;
    - per 512-col psum window and signal group, 7 accumulated matmuls
      (lhsT = xT_j, rhs = a contiguous THW column span) produce y exactly
      for taps < 512; there is no cross-window state recurrence, so every
      window is schedulable independently;
    - warm-up matmuls on a zeroed tile bridge the initial DMA wait so the
      PE's DVFS ramp (max clock after ~3us of gap-free execution) completes
      before the real stream starts.
  y is written back in a partition-major bf16 layout (flat 1KB DMA lines)
  and un-permuted + upcast on the host.  2048 signals are sharded 256 per
  NeuronCore (two groups of 128 output partitions).
"""

import ml_dtypes
import numpy as np

import concourse.bass as bass
import concourse.tile as tile
from concourse import bacc
from concourse import mybir
from concourse.bass_utils import run_bass_kernel_spmd

FP32 = mybir.dt.float32
BF16 = mybir.dt.bfloat16
FP8 = mybir.dt.float8e4
NPBF16 = ml_dtypes.bfloat16
NPF8 = ml_dtypes.float8_e4m3
F8_XS = 0.125      # fp8 operand scales: x * F8_XS, tables / F8_XS (exact)
F8_PAIRS = [(4 * w - 3, 4 * w - 2) for w in range(1, 8)]  # q=-3,-2 blocks

P = 128            # partition width == time-block length
T = 4096
NCORES = 8
NSIG = 2048        # 32*64 independent signals
SPC = NSIG // NCORES   # 256 signals per core
NST = 8            # state dim of the 4-biquad cascade
R = 4              # blocks per window
W = P * R          # 512 time steps per window
NW = T // W        # 8 windows


# ----------------------------------------------------------------------------
# host-side: derive block-filter matrices from sos
# ----------------------------------------------------------------------------

def _build_system(sos):
    """Cascade of biquads (DF2T) -> single state space (A, B, C, D), float64."""
    sos = np.asarray(sos, dtype=np.float64)
    A = np.zeros((0, 0))
    B = np.zeros((0,))
    C = np.zeros((0,))
    D = 1.0
    for (b0, b1, b2, _one, a1, a2) in sos:
        As = np.array([[-a1, 1.0], [-a2, 0.0]])
        Bs = np.array([b1 - a1 * b0, b2 - a2 * b0])
        Cs = np.array([1.0, 0.0])
        Ds = b0
        n = A.shape[0]
        Anew = np.zeros((n + 2, n + 2))
        Anew[:n, :n] = A
        Anew[n:, :n] = np.outer(Bs, C)
        Anew[n:, n:] = As
        A = Anew
        B = np.concatenate([B, Bs * D])
        C = np.concatenate([Ds * C, Cs])
        D = Ds * D
    return A, B, C, D


def _balance(A, B, C):
    """Square-root balanced realization: both gramians become diagonal and
    equal, minimizing intermediate-magnitude disparity (important because
    bf16 matmul operands are rounded; unbalanced states reach |s|~650 and
    the rounding noise then dwarfs the O(1) output)."""
    P = np.outer(B, B)
    Ak = A.copy()
    for _ in range(64):
        P = P + Ak @ P @ Ak.T
        Ak = Ak @ Ak
    Q = np.outer(C, C)
    Ak = A.copy()
    for _ in range(64):
        Q = Q + Ak.T @ Q @ Ak
        Ak = Ak @ Ak
    Rc = np.linalg.cholesky(P + 1e-30 * np.eye(len(B)))
    M = Rc.T @ Q @ Rc
    lam, U = np.linalg.eigh(M)
    lam = np.maximum(lam, 1e-30)
    Tm = Rc @ U @ np.diag(lam ** -0.25)
    Ti = np.diag(lam ** 0.25) @ U.T @ np.linalg.inv(Rc)
    return Ti @ A @ Tm, Ti @ B, C @ Tm


def _build_matrices(sos):
    """Window-fused operator tables, float64 -> caller casts to bf16.

    THW[128, 512]: cols [128d:128d+128] = Th (d=0) or (Z A_L^(d-1) F)^T (d>=1)
    ZA [8, 512]:   cols [128r:128r+128] = (Z A_L^r)^T
    FTR[128, 32]:  cols [8r:8r+8]       = ((A_L^(R-1-r)) F)^T
    A4T[8, 8]:     (A_L^R)^T
    """
    A, B, C, D = _build_system(sos)
    A, B, C = _balance(A, B, C)
    ns = A.shape[0]
    assert ns == NST

    h = np.zeros(P)
    h[0] = D
    An = np.eye(ns)
    for k in range(1, P):
        h[k] = C @ An @ B
        An = An @ A
    Th = np.zeros((P, P))
    for m in range(P):
        Th[m, m:] = h[: P - m]

    Z = np.zeros((P, ns))
    CAn = C.copy()
    for n in range(P):
        Z[n] = CAn
        CAn = CAn @ A

    F = np.zeros((ns, P))
    AmB = B.copy()
    for m in range(P - 1, -1, -1):
        F[:, m] = AmB
        AmB = A @ AmB

    AL = np.linalg.matrix_power(A, P)

    THW = np.zeros((P, R * P))
    THW[:, :P] = Th
    for d in range(1, R):
        THW[:, d * P:(d + 1) * P] = (Z @ np.linalg.matrix_power(AL, d - 1) @ F).T
    ZA = np.zeros((ns, R * P))
    for r in range(R):
        ZA[:, r * P:(r + 1) * P] = (Z @ np.linalg.matrix_power(AL, r)).T
    FTR = np.zeros((P, R * NST))
    for r in range(R):
        FTR[:, r * NST:(r + 1) * NST] = (np.linalg.matrix_power(AL, R - 1 - r) @ F).T
    A4T = np.linalg.matrix_power(AL, R).T
    return THW, ZA, FTR, A4T


# ----------------------------------------------------------------------------
# device kernel
# ----------------------------------------------------------------------------

# x chunk split (windows per DMA) interleaved across the two HWDGE engines:
# sync gets w0 alone so the first conv can start ASAP.
# x chunks at 128-time-step block granularity (32 blocks total; block
# b = window b//4, intra-window r = b%4), interleaved across the two HWDGE
# engines.  Window 0 is split across both so its data lands soonest.
XCHUNKS = [  # (engine_idx, b_lo, b_hi)
    (0, 0, 2),     # w0 r0-r1
    (1, 4, 8),     # w1 (needed ~1.4us after w0 starts -- goes first)
    (1, 2, 4),     # w0 r2-r3
    (0, 8, 12),    # w2
    (1, 12, 20),   # w3-w4
    (0, 20, 28),   # w5-w6
    (1, 28, 32),   # w7
]


def _build_nc():
    nc = bacc.Bacc("TRN2", target_bir_lowering=False)
    # xt layout: [128 tpos, (8 w, 4 r, 256 s)]  -- element [p, w, r, s]
    xt_d = nc.dram_tensor("xt", [P, NW * R * SPC], BF16, kind="ExternalInput").ap()
    ctab_d = nc.dram_tensor("ctab", [P, R * P], BF16, kind="ExternalInput").ap()
    # fp8 operands for the deep-tail (q=-3,-2) DoubleRow matmuls
    x8_d = nc.dram_tensor("x8", [P, len(F8_PAIRS) * 2 * SPC], FP8,
                          kind="ExternalInput").ap()
    ftab8_d = nc.dram_tensor("ftab8", [P, 2 * 2 * P], FP8,
                             kind="ExternalInput").ap()
    # y layout: [128 ps, (8 w, 2 g, 512 c)] -- element [p, w, g, c]
    y_d = nc.dram_tensor("y", [P, NW * 2 * W], BF16, kind="ExternalOutput").ap()

    NWARM = 15  # p-state warm-up matmuls bridging the initial DMA wait

    WCOL = R * SPC  # xt columns per window (1024)

    with tile.TileContext(nc) as tc:
        with (
            tc.tile_pool(name="consts", bufs=1) as consts,
            tc.tile_pool(name="ypool", bufs=3) as ypool,
            tc.tile_pool(name="py", bufs=3, space="PSUM") as pyp,
            tc.tile_pool(name="pw", bufs=1, space="PSUM") as pwp,
        ):
            dma_eng = (nc.sync, nc.scalar)
            # constant tables first (tiny; they gate the first matmuls),
            # then the x window chunks
            thw_sb = consts.tile([P, R * P], BF16)
            nc.sync.dma_start(thw_sb, ctab_d)
            ftab8_sb = consts.tile([P, 2, 2 * P], FP8)
            nc.sync.dma_start(ftab8_sb, ftab8_d)
            x8_sb = consts.tile([P, len(F8_PAIRS), 2, SPC], FP8)
            xblk = [[None] * R for _ in range(NW)]
            for eng, b_lo, b_hi in XCHUNKS:
                t = consts.tile([P, (b_hi - b_lo) * SPC], BF16, name=f"xb{b_lo}")
                dma_eng[eng].dma_start(
                    t, xt_d[:, b_lo * SPC:b_hi * SPC]
                )
                for b in range(b_lo, b_hi):
                    xblk[b // R][b % R] = t[:, (b - b_lo) * SPC:(b - b_lo + 1) * SPC]
                if b_lo == 4:  # x8 needed from window 1; queue it right after
                    nc.scalar.dma_start(x8_sb, x8_d)

            # warm-up: keep the PE continuously busy through the DVFS ramp
            # (max clock needs ~3us of gap-free execution) while the first
            # x/ctab DMAs are in flight.  Zeroed operands, result unused.
            warm_sb = consts.tile([P, 3 * P], BF16, name="warm")
            nc.gpsimd.memset(warm_sb, 0)
            psum_warm = pwp.tile([P, 3 * P], FP32, tag="warm")
            for _ in range(NWARM):
                nc.tensor.matmul(
                    psum_warm, warm_sb[:, 0:P], warm_sb, start=True, stop=True,
                )

            for w in range(NW):
                psum_y = [
                    pyp.tile([P, W], FP32, tag=f"py{g}", name=f"py{g}")
                    for g in (0, 1)
                ]
                y_sb = ypool.tile([P, 2 * W], BF16, tag="y", name="y_sb")

                # terms (j, out span c0:c1, table span t0:t1); j = 4w (the
                # d=0 full-span term) goes first so its start=True resets
                # every psum column before the others accumulate
                terms = []
                q_lo = -1 if w >= 1 else 0  # q<=-2 goes via the fp8 DoubleRow
                for j in range(R * w + q_lo, R * w + R):
                    o_lo, o_hi = max(R * w, j), min(R * w + R - 1, j + R - 1)
                    terms.append((j, (o_lo - R * w) * P, (o_hi - R * w + 1) * P,
                                  (o_lo - j) * P, (o_hi - j + 1) * P))
                terms.sort(key=lambda e: (e[0] != R * w, -(e[2] - e[1])))

                def y_group(g):
                    for i, (j, c0, c1, t0, t1) in enumerate(terms):
                        nc.tensor.matmul(
                            psum_y[g][:, c0:c1],
                            xblk[j // R][j % R][:, g * P:(g + 1) * P],
                            thw_sb[:, t0:t1],
                            start=(i == 0),
                            stop=(w == 0 and i == len(terms) - 1),
                        )
                    if w >= 1:
                        # q=-3 and q=-2 tail terms fused into one fp8
                        # DoubleRow matmul (0.5 cyc/row): out[:, 0:256] +=
                        # x_{4w-3}^T @ T3pad + x_{4w-2}^T @ T2
                        nc.tensor.matmul(
                            psum_y[g][:, 0:2 * P],
                            x8_sb[:, w - 1, :, g * P:(g + 1) * P],
                            ftab8_sb,
                            start=False, stop=True,
                            perf_mode=mybir.MatmulPerfMode.DoubleRow,
                        )

                y_group(0)
                y_group(1)

                # psum -> sbuf (bf16) -> DRAM, halves on separate engines;
                # last window: each psum is half-copied by BOTH copy engines
                # and stored immediately, so the final store chain is short
                if w == NW - 1:
                    H = W // 2
                    for g in (0, 1):
                        for h, ceng in ((0, nc.vector.tensor_copy),
                                        (1, nc.scalar.copy)):
                            c0 = g * W + h * H
                            ceng(y_sb[:, c0:c0 + H],
                                 psum_y[g][:, h * H:(h + 1) * H])
                            dma_eng[h].dma_start(
                                y_d[:, w * 2 * W + c0: w * 2 * W + c0 + H],
                                y_sb[:, c0:c0 + H],
                            )
                else:
                    # mid-kernel stores go out via SWDGE (gpsimd) so the two
                    # HWDGE sequencers never queue descriptor-gen work behind
                    # the latency-critical last-window copies/stores
                    nc.vector.tensor_copy(y_sb[:, 0:W], psum_y[0])
                    nc.gpsimd.dma_start(
                        y_d[:, w * 2 * W: w * 2 * W + W], y_sb[:, 0:W]
                    )
                    nc.scalar.copy(y_sb[:, W:2 * W], psum_y[1])
                    nc.gpsimd.dma_start(
                        y_d[:, w * 2 * W + W:(w + 1) * 2 * W], y_sb[:, W:2 * W]
                    )
    nc.compile()
    return nc


_NC_CACHE = None
LAST_RESULTS = None  # BassKernelResults of the most recent kernel() call


def _get_nc():
    global _NC_CACHE
    if _NC_CACHE is None:
        _NC_CACHE = _build_nc()
    return _NC_CACHE


def kernel(x: np.ndarray, sos: np.ndarray) -> np.ndarray:
    x = np.asarray(x)
    orig_shape = x.shape
    orig_dtype = x.dtype
    THW, ZA, FTR, A4T = _build_matrices(np.asarray(sos, dtype=np.float64))

    bf = lambda a: np.ascontiguousarray(np.asarray(a, dtype=NPBF16))
    ctab = bf(THW)
    ftab8 = np.zeros((P, 2, 2 * P), np.float64)
    ftab8[:, 0, 0:P] = THW[:, 3 * P:] / F8_XS      # q=-3 table, zero-padded
    ftab8[:, 1, :] = THW[:, 2 * P:] / F8_XS        # q=-2 table
    ftab8 = np.ascontiguousarray(ftab8.reshape(P, 4 * P).astype(NPF8))

    # [core, sig, w, r, p] -> [core, p, w, r, sig]
    xr = x.reshape(NCORES, SPC, NW, R, P).transpose(0, 4, 2, 3, 1)
    xtb = bf(xr)
    xt = xtb.reshape(NCORES, P, NW * R * SPC)
    xb32 = np.asarray(xtb, np.float32).reshape(NCORES, P, NW * R, SPC)
    idx = [j for pair in F8_PAIRS for j in pair]
    x8 = np.ascontiguousarray(
        (xb32[:, :, idx, :] * F8_XS).astype(NPF8)
    ).reshape(NCORES, P, len(idx) * SPC)

    in_maps = [
        {"xt": xt[c], "ctab": ctab, "x8": x8[c], "ftab8": ftab8}
        for c in range(NCORES)
    ]
    nc = _get_nc()
    res = run_bass_kernel_spmd(nc, in_maps, core_ids=list(range(NCORES)))
    global LAST_RESULTS
    LAST_RESULTS = res
    # y_d [128 p, 8 w, 2 g, 512 c] -> y[core, g*128+p, w*512+c]
    y = np.stack([
        np.asarray(res.results[c]["y"])
        .reshape(P, NW, 2, W)
        .transpose(2, 0, 1, 3)
        .reshape(SPC, T)
        for c in range(NCORES)
    ])
    return y.reshape(orig_shape).astype(orig_dtype, copy=False)
